# revision 35
# baseline (speedup 1.0000x reference)
"""Trainium2 Bass kernel: LSTM encoder (scan LSTMCell over T, return final carry).

B=64, T=1024, F=H=512.  Data-parallel over batch: core c runs the full
recurrence for batch half c%2 (cores 2-7 run redundant work, discarded).

Per core, per step: gates = [h;x_t;1] @ [Wh;Wx;b] as one fused PSUM
accumulation using 4-way column-tiled bf16 matmuls issued k-outer/strip-inner
(so the 4 concurrent PE column strips never head-of-line block the in-order
PE queue), with the 4H dimension permuted into 4 bands: band q holds gates
[i f g o] of hidden quarter q on psum partitions [32q, 32q+32).

The pointwise chain uses sigmoid(x) = (tanh(x/2)+1)/2 for ALL sigmoid gates
(i, f, o) with the 1/2 pre-scales folded into the packed weights and a
doubled cell state C = 2c, kept inside the th tile ([i|f|g|C] cols 0:512,
tanh_o in its own bf16 tile) so the cell update is minimal fused DVE ops:
  th_ifg = tanh(gates_ifg'); th_o = tanh(gates_o')   (ACT x2)
  [t1|u] = (th[i|f]+1)*(th[g|C])   (ONE stt, DVE)  [= 2ig | 4fc]
  C      = u*0.5 + t1              (stt, DVE)      [= 2c_new]
  then split in two 64-col halves so hT cols 0:64 unblock h-chunks 0,1 early:
  tc = tanh(C*0.5); ho = (th_o+1)*tc -> bf16; hT half = vector.transpose(ho)
The stationary operand hT is produced WITHOUT any PE transpose: Wh's rows are
permuted so that stationary chunk j = h-dims {128q+32j+c}, which makes the
DVE 32x32 block transpose of ho (band-batch x quarter layout) directly a
valid lhsT.  The PE queue carries pure matmuls; the x-projection matmuls are
issued two steps ahead of use so the PE stays busy (and the HAM clock stays
warm, avoiding K=4/8 re-throttle) while the pointwise chain runs.

The serial per-step cycle h_mms -> tanh -> t1u -> C -> tc -> ho -> hT is the
bound (~3.8us/step measured; engines are otherwise <60% busy).  An int32
input r repeats the whole scan (r=1 computes the real output; r>1 is used
for differential on-device timing).
"""
import sys
sys.path.insert(0, '/opt/trn_rl_repo')
import numpy as np
from ml_dtypes import bfloat16
import concourse.bass as bass
import concourse.mybir as mybir
import concourse.tile as tile
import concourse.bacc as bacc
from concourse.bass_utils import run_bass_kernel_spmd

DT = mybir.dt.float32
DTM = mybir.dt.bfloat16
B2, H, F, T_FULL = 32, 512, 512, 1024
UB = 16          # steps per hardware-loop body
NS = 1           # batch-half streams per core (core c handles half c%2)
N_CORES = 8


def _pack_weights(Wh, Wx, b):
    Wh = np.asarray(Wh, np.float32)
    Wx = np.asarray(Wx, np.float32)
    b = np.asarray(b, np.float32)
    # raw gate order i,f,g,o; tanh-trick pre-scale 0.5 on i, f and o; Wh acts on 2h
    scale_gate = np.concatenate([
        np.full(H, 0.5, np.float32), np.full(H, 0.5, np.float32),
        np.ones(H, np.float32), np.full(H, 0.5, np.float32)])
    Whs = Wh * 0.5 * scale_gate[None, :]
    Wxs = Wx * scale_gate[None, :]
    bs = b * scale_gate
    # Row-permute Wh so stationary chunk j = h-dims {128q+32j+c}: this makes
    # the DVE 32x32 block transpose of ho (band-batch x quarter layout)
    # directly produce a valid lhsT, with no PE transpose.
    perm_rows = np.array([128 * q + 32 * k + c
                          for k in range(4) for q in range(4) for c in range(32)])
    Whs = Whs[perm_rows, :]
    W_cat = np.concatenate([Whs, Wxs, bs[None, :]], 0)  # [1025, 4H]
    perm = []
    for q in range(4):
        for gate in (0, 1, 2, 3):  # band column order [i, f, g, o]
            perm += list(range(gate * H + 128 * q, gate * H + 128 * q + 128))
    W_pad = np.zeros((9 * 128, 4 * H), np.float32)
    W_pad[:2 * H + 1] = W_cat[:, perm]
    return W_pad.reshape(9, 128, 4 * H).astype(bfloat16)


def _unband(r):
    return np.concatenate([r[32 * q:32 * q + B2] for q in range(4)], axis=1)


def _build(T, max_repeat=64):
    assert T % UB == 0 and UB % 2 == 0
    AluOp = mybir.AluOpType
    nc = bacc.Bacc("TRN2", target_bir_lowering=False, debug=False)
    x_d = [nc.dram_tensor(f"x{s}", [B2, T + UB, F], DTM, kind="ExternalInput")
           for s in range(NS)]
    W_d = nc.dram_tensor("W", [9, 128, 4 * H], DTM, kind="ExternalInput")
    ones_d = nc.dram_tensor("ones", [1, 32], DTM, kind="ExternalInput")
    r_d = nc.dram_tensor("r", [1, 1], mybir.dt.int32, kind="ExternalInput")
    cO_d = [nc.dram_tensor(f"c_out{s}", [128, 128], DT, kind="ExternalOutput")
            for s in range(NS)]
    hO_d = [nc.dram_tensor(f"h_out{s}", [128, 128], DT, kind="ExternalOutput")
            for s in range(NS)]

    AF = mybir.ActivationFunctionType
    HB = UB // 8
    with tile.TileContext(nc) as tc:
        with tc.tile_pool(name="sb", bufs=1) as pool, \
             tc.tile_pool(name="ps", bufs=1, space="PSUM") as pps:
            W_s = pool.tile([128, 9 * 2048], DTM, name="W_s")
            ones_s = pool.tile([1, 32], DTM, name="ones_s")
            r_s = pool.tile([1, 1], mybir.dt.int32, name="r_s")
            # th layout per stream: [ tanh_i | tanh_f | tanh_g | C=2c ]
            # (cols 0:128, 128:256, 256:384, 384:512); tanh_o lives in tho
            # (bf16, so the ho stt runs in DVE 2x mode)
            hT = [[pool.tile([128, 128], DTM, name=f"hT{s}_{p}") for p in range(2)]
                  for s in range(NS)]
            xst = [[pool.tile([128, 1024], DTM, name=f"xst{s}_{i}") for i in range(HB)]
                   for s in range(NS)]
            th_s = [pool.tile([128, 512], DT, name=f"th{s}") for s in range(NS)]
            tho_s = [pool.tile([128, 128], DTM, name=f"tho{s}") for s in range(NS)]
            t1u_s = [pool.tile([128, 256], DT, name=f"t1u{s}") for s in range(NS)]
            tc_s = [pool.tile([128, 128], DTM, name=f"tc{s}") for s in range(NS)]
            ho_s = [pool.tile([128, 128], DTM, name=f"ho{s}") for s in range(NS)]
            h2_s = [pool.tile([128, 128], DT, name=f"h2_{s}") for s in range(NS)]
            gates_p = [[pps.tile([128, 512], DT, name=f"g{s}_{j}") for j in range(4)]
                       for s in range(NS)]

            for k in range(9):
                nc.sync.dma_start(W_s[:, 2048 * k:2048 * (k + 1)], W_d.ap()[k])
            nc.sync.dma_start(ones_s[:], ones_d.ap()[:])
            nc.sync.dma_start(r_s[:], r_d.ap()[:])
            R = nc.values_load(r_s[:], min_val=1, max_val=max_repeat,
                               skip_runtime_bounds_check=True)

            def xtile_k(s, j, k):
                i, t = divmod(j, 8)
                e = 4 * t + k
                return xst[s][i][:, 32 * e:32 * e + 32]

            def x_mms(s, j, bank):
                out = gates_p[s][bank]
                for n, k in enumerate([4, 5, 6, 7]):
                    for g4 in range(4):
                        nc.tensor.matmul(out[32 * g4:32 * g4 + 32, :], xtile_k(s, j, k - 4),
                                         W_s[:, 2048 * k + 512 * g4:2048 * k + 512 * g4 + 512],
                                         start=(n == 0), stop=False, tile_position=(0, 32 * g4),
                                         skip_group_check=True)
                for g4 in range(4):
                    nc.tensor.matmul(out[32 * g4:32 * g4 + 32, :], ones_s[:],
                                     W_s[0:1, 2048 * 8 + 512 * g4:2048 * 8 + 512 * g4 + 512],
                                     start=False, stop=False, tile_position=(0, 32 * g4),
                                     skip_group_check=True)

            def h_mms(s, par, bank):
                # igf columns (0:384) first so their tanh starts ~500ns before
                # the o columns (384:512) finish
                out = gates_p[s][bank]
                for n, k in enumerate([0, 1, 2, 3]):
                    for g4 in range(4):
                        base = 2048 * k + 512 * g4
                        nc.tensor.matmul(out[32 * g4:32 * g4 + 32, :],
                                         hT[s][par][:, 32 * k:32 * k + 32],
                                         W_s[:, base:base + 512],
                                         start=False, stop=(n == 3), tile_position=(0, 32 * g4),
                                         skip_group_check=True)

            def chain(s, par, bank):
                th = th_s[s]; t1u = t1u_s[s]; tho = tho_s[s]
                tcs = tc_s[s]; ho = ho_s[s]
                g = gates_p[s][bank]
                npar = 1 - par
                nc.scalar.activation(th[:, 0:384], g[:, 0:384], AF.Tanh)
                # [t1 | u] = [(th_i+1)*th_g | (th_f+1)*C] in one stt
                nc.vector.scalar_tensor_tensor(t1u[:], th[:, 0:256], 1.0, th[:, 256:512],
                                               op0=AluOp.add, op1=AluOp.mult)
                nc.scalar.activation(tho[:], g[:, 384:512], AF.Tanh)
                # C halves so tc_a can start off C_a early
                for c0, c1 in ((0, 64), (64, 128)):
                    nc.vector.scalar_tensor_tensor(th[:, 384 + c0:384 + c1],
                                                   t1u[:, 128 + c0:128 + c1], 0.5,
                                                   t1u[:, c0:c1],
                                                   op0=AluOp.mult, op1=AluOp.add)
                # tail split in two 64-col halves: hT cols 0:64 (h-chunks 0,1)
                # ready ~350ns before cols 64:128 (h-chunks 2,3)
                for c0, c1 in ((0, 64), (64, 128)):
                    nc.scalar.activation(tcs[:, c0:c1], th[:, 384 + c0:384 + c1],
                                         AF.Tanh, scale=0.5)
                    nc.vector.scalar_tensor_tensor(ho[:, c0:c1], tho[:, c0:c1],
                                                   1.0, tcs[:, c0:c1],
                                                   op0=AluOp.add, op1=AluOp.mult)
                    nc.vector.transpose(hT[s][npar][:, c0:c1], ho[:, c0:c1])

            def stage_dma(s, i, t_expr):
                src = x_d[s].ap()[:, bass.ds(t_expr, 8), :].rearrange("b t f -> b (t f)")
                nc.sync.dma_start_transpose(
                    xst[s][i][:].rearrange("p (e b) -> p e b", b=32), src)

            nbody = T // UB
            with tc.For_i(0, R) as rep:
                for s in range(NS):
                    nc.vector.memset(th_s[s][:, 384:512], 0.0)
                    for p in range(2):
                        nc.vector.memset(hT[s][p][:], 0.0)
                    for i in range(HB):
                        stage_dma(s, i, 8 * i)
                    x_mms(s, 0, 0)
                    x_mms(s, 1, 1)
                with tc.For_i(0, nbody) as tb:
                    next_t0 = tb * UB + UB
                    for j in range(UB):
                        sj = j % 4
                        par = j % 2
                        for s in range(NS):
                            h_mms(s, par, sj)
                            x_mms(s, (j + 2) % UB, (sj + 2) % 4)
                            if j % 8 == 7:
                                stage_dma(s, j // 8, next_t0 + 8 * (j // 8))
                            chain(s, par, sj)

            # outputs: c = C/2 and h = h2/2 recovered on host (fp32 tail)
            for s in range(NS):
                nc.vector.tensor_scalar_add(h2_s[s][:], tho_s[s][:], 1.0)
                nc.vector.tensor_mul(h2_s[s][:], h2_s[s][:], tc_s[s][:])
                nc.sync.dma_start(cO_d[s].ap()[:], th_s[s][:, 384:512])
                nc.sync.dma_start(hO_d[s].ap()[:], h2_s[s][:])

    nc.finalize()
    return nc


_NC_CACHE = {}


def kernel(inputs, Wx, Wh, b):
    x = np.asarray(inputs, np.float32)
    Wx = np.asarray(Wx, np.float32)
    Wh = np.asarray(Wh, np.float32)
    b = np.asarray(b, np.float32)
    Bf, T, _ = x.shape
    assert Bf == 2 * B2
    if T not in _NC_CACHE:
        _NC_CACHE[T] = _build(T)
    nc = _NC_CACHE[T]

    W = _pack_weights(Wh, Wx, b)
    ones = np.ones((1, 32), bfloat16)
    r1 = np.array([[1]], np.int32)
    xb = np.zeros((Bf, T + UB, x.shape[2]), bfloat16)
    xb[:, :T] = x.astype(bfloat16)
    halves = [np.ascontiguousarray(xb[:B2]), np.ascontiguousarray(xb[B2:])]
    in_maps = [{"x0": halves[core % 2], "W": W, "ones": ones, "r": r1}
               for core in range(N_CORES)]
    res = run_bass_kernel_spmd(nc, in_maps, list(range(N_CORES)))
    c = np.concatenate([_unband(res.results[0]["c_out0"]),
                        _unband(res.results[1]["c_out0"])], 0) * 0.5
    h = np.concatenate([_unband(res.results[0]["h_out0"]),
                        _unband(res.results[1]["h_out0"])], 0) * 0.5
    return c, h
